# revision 20
# baseline (speedup 1.0000x reference)
"""Trainium2 Bass kernel for nn_AdaptiveWaveletBank.

out[b, s, n] = sum_k w_s[k] * signal[b, n - wl_s + k]   (complex w, zero-pad)

Strategy:
  - Data-parallel over batch: 16 rows -> 8 cores x 2 rows.
  - The Morlet-like wavelet w_s decays as exp(-0.5 (k/scale)^2): only the
    first ~6.1*scale taps matter (<1e-8 of peak).  Host truncates.
  - Per core the conv is expressed as banded matmuls on the TensorEngine:
    signal laid out as tiles S[j, m] = sig[m*128 + j]; for each output
    half (128 tiles x 128 samples) and scale, accumulate over tile-shifts t:
        out[c, 2u+comp] += sum_j S[j, c + c0 - t] * A_{s,t}[j, 2u+comp]
    with A_{s,t}[j, 2u+comp] = w_comp[wl_s + j - u - 128 t] (banded Toeplitz,
    built on host, fp16).  PSUM accumulates in fp32; DVE copies/casts to an
    fp16 staging buffer laid out so the output DMA is fully contiguous.
  - Host pre-transposes/pads the signal and assembles complex64 output.
"""

import numpy as np

import concourse.bacc as bacc
import concourse.bass as bass
import concourse.mybir as mybir
import concourse.tile as tile
from concourse.bass_utils import run_bass_kernel_spmd

B, L, NSC = 16, 32768, 16
NCORES = 8
ROWS = B // NCORES          # rows of the batch per core
NT = L // 128               # 256 signal tiles of 128 samples
PAD = 16                    # leading zero tiles (max tile shift t = 16)
NUM_OSC = 6.0
ENV_CUT = 1e-8              # truncate wavelet where envelope < this

F16 = mybir.dt.float16
F32 = mybir.dt.float32


def _scales_and_lengths():
    s = np.exp(np.linspace(np.log(1.0), np.log(32.0), NSC))
    lengths = []
    for sc in s:
        wl = min(int(L * 0.5), int(64 * sc))
        wl = max(wl, 8)
        wl = wl if wl % 2 == 0 else wl + 1
        lengths.append(wl)
    return s, lengths


def _wavelets(sc, wl, cf, bw):
    # float32 arithmetic to mirror the jnp reference
    t = np.arange(wl, dtype=np.float32) / (bw * np.float32(max(float(sc), 0.1)))
    env = np.exp(-0.5 * t * t).astype(np.float32)
    ph = (np.float32(2.0 * np.pi / NUM_OSC) * cf * t).astype(np.float32)
    wr = env * np.cos(ph)
    wi = env * np.sin(ph)
    norm = np.max(np.sqrt(wr * wr + wi * wi)) + np.float32(1e-8)
    return (wr / norm).astype(np.float32), (wi / norm).astype(np.float32), env


def _plan(cf, bw):
    """Per-scale wavelets, truncation and tile-shift ranges + packed A matrix.

    Each scale picks a signal-tiling phase (0 or 64 samples) minimizing the
    number of tile-shift matmuls nT."""
    s_vals, wlens = _scales_and_lengths()
    scales = []
    cols = 0
    for sc, wl in zip(s_vals, wlens):
        wr, wi, env = _wavelets(sc, wl, cf, bw)
        kcut = int(np.sum(env > ENV_CUT))
        kcut = max(1, min(kcut, wl))
        pad64 = (64 - wl % 64) % 64
        if pad64 + kcut <= 64 and wl >= 64:
            # even/odd half-tile mode: two 128-col single matmuls sharing
            # one A block; G = ceil(wl/64) aligns the 128-sample window
            scales.append(dict(wl=wl, wr=wr, wi=wi, kcut=kcut, mode="eo",
                               G=-(-wl // 64), col=cols))
            cols += 128
            continue
        best = None
        for ph in (0, 64):
            t_hi = (wl - ph + 127) // 128
            t_lo = -(-(wl - ph - kcut - 126) // 128)
            if t_lo < 0 and ph > 0:
                continue          # would need a forward tile shift
            t_lo = max(0, t_lo)
            if best is None or t_hi - t_lo < best[1] - best[0]:
                best = (t_lo, t_hi, ph)
        t_lo, t_hi, ph = best
        ts = list(range(t_lo, t_hi + 1))
        scales.append(dict(wl=wl, wr=wr, wi=wi, kcut=kcut, mode="chain",
                           ts=ts, col=cols, ph=ph))
        cols += len(ts) * 256

    amat = np.zeros((128, cols), dtype=np.float16)
    j = np.arange(128)[:, None]
    for sp in scales:
        wl, wr, wi, kcut = sp["wl"], sp["wr"], sp["wi"], sp["kcut"]
        if sp["mode"] == "eo":
            u = np.arange(64)[None, :]
            k = wl - 64 * sp["G"] + j - u
            valid = (k >= 0) & (k < kcut)
            kc = np.clip(k, 0, wl - 1)
            blk = np.zeros((128, 128), dtype=np.float32)
            blk[:, 0::2] = np.where(valid, wr[kc], 0.0)
            blk[:, 1::2] = np.where(valid, wi[kc], 0.0)
            amat[:, sp["col"]:sp["col"] + 128] = blk.astype(np.float16)
            continue
        u = np.arange(128)[None, :]
        for i, t in enumerate(sp["ts"]):
            k = wl - sp["ph"] + j - u - 128 * t
            valid = (k >= 0) & (k < kcut)
            kc = np.clip(k, 0, wl - 1)
            blk = np.zeros((128, 256), dtype=np.float32)
            blk[:, 0::2] = np.where(valid, wr[kc], 0.0)
            blk[:, 1::2] = np.where(valid, wi[kc], 0.0)
            off = sp["col"] + i * 256
            amat[:, off:off + 256] = blk.astype(np.float16)
    return scales, amat


def _make_sig(sig_rows):
    """(ROWS, L) fp32 -> (ROWS, 2 phases, 128, PAD+NT) fp16 tiled/padded."""
    st = np.zeros((ROWS, 2, 128, PAD + NT), dtype=np.float16)
    s16 = sig_rows.astype(np.float16)
    for r in range(ROWS):
        st[r, 0, :, PAD:] = s16[r].reshape(NT, 128).T
        x64 = np.zeros(L, dtype=np.float16)
        x64[64:] = s16[r][:L - 64]
        st[r, 1, :, PAD:] = x64.reshape(NT, 128).T
    return st


def _build_nc(scales, acols):
    """Build + schedule + compile the per-core Bass program."""
    nc = bacc.Bacc("TRN2", target_bir_lowering=False, debug=False,
                   num_devices=NCORES)

    sig_d = nc.dram_tensor("sig", [ROWS, 2, 128, PAD + NT], F16,
                           kind="ExternalInput")
    amat_d = nc.dram_tensor("amat", [128, acols], F16, kind="ExternalInput")
    # out[row, half, c, s, 2u+comp] ; n = half*16384 + c*128 + u
    out_d = nc.dram_tensor("out", [ROWS, 2, 128, NSC, 256], F16,
                           kind="ExternalOutput")

    with tile.TileContext(nc) as tc:
        with tc.tile_pool(name="const", bufs=1) as const_pool, \
             tc.tile_pool(name="ob", bufs=8) as ob_pool, \
             tc.tile_pool(name="ps", bufs=1, space="PSUM") as ps_pool:

            # tiny ACT warm-up so the activation-table load (~2.7us)
            # happens under the input DMAs, not on the first real copy
            wz = const_pool.tile([128, 512], F16, tag="wz")
            wz2 = const_pool.tile([128, 8], F16, tag="wz2")
            nc.vector.memset(wz[:], 0)
            nc.scalar.copy(wz2[:], wz[:, 0:8])

            # amat chunked across BOTH HWDGE rings so the first matmuls only
            # wait for a small chunk; signal as one fused DMA on the sync ring
            amat_t = const_pool.tile([128, acols], F16, tag="amat")
            sig_all = const_pool.tile([128, 4 * (PAD + NT)], F16, tag="sig")

            def acol(s):
                return scales[s]["col"] if s < NSC else acols

            def amat_dma(eng, s0, s1):
                c0, c1 = acol(s0), acol(s1)
                eng.dma_start(out=amat_t[:, c0:c1],
                              in_=amat_d.ap()[:, c0:c1])

            amat_dma(nc.scalar, 0, 2)
            nc.scalar.dma_start(
                out=sig_all[:].rearrange("j (g m) -> j g m", g=4),
                in_=sig_d.ap().rearrange("r p j m -> j (r p) m"))
            amat_dma(nc.scalar, 2, 4)
            amat_dma(nc.scalar, 4, 8)
            amat_dma(nc.scalar, 8, 12)
            amat_dma(nc.scalar, 12, 16)

            def sig_slice(r, p, lo, hi):
                base = (2 * r + p) * (PAD + NT)
                return sig_all[:, base + lo:base + hi]

            # HAM warm-up: dummy matmuls keep the PE busy during the input
            # DMAs so real matmuls run at 2.4 GHz from the start
            dmy = ps_pool.tile([128, 2, 512], F32, tag="ps0")
            for _ in range(8):
                nc.tensor.matmul(dmy[:, 0, :], wz[:, 0:128], wz[:],
                                 start=True, stop=True)

            pg = 0
            for grp in range(2):
                for row in range(ROWS):
                    for half in range(2):
                        ob = ob_pool.tile([128, 8, 256], F16, tag="ob")
                        for pair in range(4):     # 2 scales per psum tile
                            pg += 1
                            ps = ps_pool.tile([128, 2, 512], F32,
                                              tag=f"ps{pg % 4}")
                            for kk in range(2):
                                s = grp * 8 + pair * 2 + kk
                                sp = scales[s]
                                if sp["mode"] == "eo":
                                    # even/odd half-tile: n = 128m + 64*eo + u
                                    for eo in range(2):
                                        Ge = sp["G"] - eo
                                        lo = PAD + 128 * half - (Ge - Ge % 2) // 2
                                        nc.tensor.matmul(
                                            ps[:, kk, eo * 128:eo * 128 + 128],
                                            sig_slice(row, Ge % 2,
                                                      lo, lo + 128),
                                            amat_t[:, sp["col"]:
                                                   sp["col"] + 128],
                                            start=True, stop=True,
                                        )
                                    continue
                                nts = len(sp["ts"])
                                for i, t in enumerate(sp["ts"]):
                                    lo = PAD + 128 * half - t
                                    nc.tensor.matmul(
                                        ps[:, kk, 0:256],
                                        sig_slice(row, sp["ph"] // 64,
                                                  lo, lo + 128),
                                        amat_t[:, sp["col"] + i * 256:
                                               sp["col"] + (i + 1) * 256],
                                        start=(i == 0),
                                        stop=(i == nts - 1),
                                    )
                            # per-scale copies split across both engines
                            nc.vector.tensor_copy(ob[:, pair * 2, :],
                                                  ps[:, 0, 0:256])
                            nc.scalar.copy(ob[:, pair * 2 + 1, :],
                                           ps[:, 1, 0:256])
                        last = (grp == 1 and row == ROWS - 1 and half == 1)
                        if not last:
                            dma_eng = nc.sync if (row + half) % 2 == 0 \
                                else nc.scalar
                            dma_eng.dma_start(
                                out=out_d.ap()[row, half, :,
                                               grp * 8:(grp + 1) * 8, :]
                                    .rearrange("c s i -> c (s i)"),
                                in_=ob[:].rearrange("c s i -> c (s i)"),
                            )
                        else:
                            # final store split across both rings so the
                            # kernel tail is half the size
                            for q, eng in ((0, nc.sync), (1, nc.scalar)):
                                s0 = grp * 8 + q * 4
                                eng.dma_start(
                                    out=out_d.ap()[row, half, :,
                                                   s0:s0 + 4, :]
                                        .rearrange("c s i -> c (s i)"),
                                    in_=ob[:, q * 4:(q + 1) * 4, :]
                                        .rearrange("c s i -> c (s i)"),
                                )
    nc.compile()
    return nc


_CACHE = {}


def _get_nc(key, scales, acols):
    if key not in _CACHE:
        _CACHE[key] = _build_nc(scales, acols)
    return _CACHE[key]


def kernel(signal, scales_log, center_freq_log, bandwidth_log):
    signal = np.asarray(signal, dtype=np.float32)
    cf = np.float32(np.exp(np.float32(np.asarray(center_freq_log))))
    bw = np.float32(np.exp(np.float32(np.asarray(bandwidth_log))))

    scales, amat = _plan(cf, bw)
    key = tuple((sp["mode"], sp["col"], sp.get("G", -1),
                 tuple(sp.get("ts", ())), sp.get("ph", -1))
                for sp in scales)
    nc = _get_nc(key, scales, amat.shape[1])

    # host: per-core transposed/padded fp16 signal
    in_maps = []
    for core in range(NCORES):
        st = _make_sig(signal[core * ROWS:(core + 1) * ROWS])
        in_maps.append({"sig": st, "amat": amat})

    res = run_bass_kernel_spmd(nc, in_maps, core_ids=list(range(NCORES)))

    out = np.empty((B, NSC, L), dtype=np.complex64)
    for core in range(NCORES):
        o = np.asarray(res.results[core]["out"], dtype=np.float32)
        # [row, half, c, s, 2u+comp] -> [row, s, half, c, u, comp]
        o = o.transpose(0, 3, 1, 2, 4).reshape(ROWS, NSC, L, 2)
        out[core * ROWS:(core + 1) * ROWS] = o[..., 0] + 1j * o[..., 1]
    return out


# revision 21
# speedup vs baseline: 1.0724x; 1.0724x over previous
"""Trainium2 Bass kernel for nn_AdaptiveWaveletBank.

out[b, s, n] = sum_k w_s[k] * signal[b, n - wl_s + k]   (complex w, zero-pad)

Strategy:
  - Data-parallel over batch: 16 rows -> 8 cores x 2 rows.
  - The Morlet-like wavelet w_s decays as exp(-0.5 (k/scale)^2): only the
    first ~6.1*scale taps matter (<1e-8 of peak).  Host truncates.
  - Per core the conv is expressed as banded matmuls on the TensorEngine:
    signal laid out as tiles S[j, m] = sig[m*128 + j]; for each output
    half (128 tiles x 128 samples) and scale, accumulate over tile-shifts t:
        out[c, 2u+comp] += sum_j S[j, c + c0 - t] * A_{s,t}[j, 2u+comp]
    with A_{s,t}[j, 2u+comp] = w_comp[wl_s + j - u - 128 t] (banded Toeplitz,
    built on host, fp16).  PSUM accumulates in fp32; DVE copies/casts to an
    fp16 staging buffer laid out so the output DMA is fully contiguous.
  - Host pre-transposes/pads the signal and assembles complex64 output.
"""

import numpy as np

import concourse.bacc as bacc
import concourse.bass as bass
import concourse.mybir as mybir
import concourse.tile as tile
from concourse.bass_utils import run_bass_kernel_spmd

B, L, NSC = 16, 32768, 16
NCORES = 8
ROWS = B // NCORES          # rows of the batch per core
NT = L // 128               # 256 signal tiles of 128 samples
PAD = 16                    # leading zero tiles (max tile shift t = 16)
NUM_OSC = 6.0
ENV_CUT = 1e-8              # truncate wavelet where envelope < this

F16 = mybir.dt.float16
F32 = mybir.dt.float32


def _scales_and_lengths():
    s = np.exp(np.linspace(np.log(1.0), np.log(32.0), NSC))
    lengths = []
    for sc in s:
        wl = min(int(L * 0.5), int(64 * sc))
        wl = max(wl, 8)
        wl = wl if wl % 2 == 0 else wl + 1
        lengths.append(wl)
    return s, lengths


def _wavelets(sc, wl, cf, bw):
    # float32 arithmetic to mirror the jnp reference
    t = np.arange(wl, dtype=np.float32) / (bw * np.float32(max(float(sc), 0.1)))
    env = np.exp(-0.5 * t * t).astype(np.float32)
    ph = (np.float32(2.0 * np.pi / NUM_OSC) * cf * t).astype(np.float32)
    wr = env * np.cos(ph)
    wi = env * np.sin(ph)
    norm = np.max(np.sqrt(wr * wr + wi * wi)) + np.float32(1e-8)
    return (wr / norm).astype(np.float32), (wi / norm).astype(np.float32), env


def _plan(cf, bw):
    """Per-scale wavelets, truncation and tile-shift ranges + packed A matrix.

    Each scale picks a signal-tiling phase (0 or 64 samples) minimizing the
    number of tile-shift matmuls nT."""
    s_vals, wlens = _scales_and_lengths()
    scales = []
    cols = 0
    for sc, wl in zip(s_vals, wlens):
        wr, wi, env = _wavelets(sc, wl, cf, bw)
        kcut = int(np.sum(env > ENV_CUT))
        kcut = max(1, min(kcut, wl))
        pad64 = (64 - wl % 64) % 64
        if pad64 + kcut <= 64 and wl >= 64:
            # even/odd half-tile mode: two 128-col single matmuls sharing
            # one A block; G = ceil(wl/64) aligns the 128-sample window
            scales.append(dict(wl=wl, wr=wr, wi=wi, kcut=kcut, mode="eo",
                               G=-(-wl // 64), col=cols))
            cols += 128
            continue
        best = None
        for ph in (0, 64):
            t_hi = (wl - ph + 127) // 128
            t_lo = -(-(wl - ph - kcut - 126) // 128)
            if t_lo < 0 and ph > 0:
                continue          # would need a forward tile shift
            t_lo = max(0, t_lo)
            if best is None or t_hi - t_lo < best[1] - best[0]:
                best = (t_lo, t_hi, ph)
        t_lo, t_hi, ph = best
        ts = list(range(t_lo, t_hi + 1))
        scales.append(dict(wl=wl, wr=wr, wi=wi, kcut=kcut, mode="chain",
                           ts=ts, col=cols, ph=ph))
        cols += len(ts) * 256

    amat = np.zeros((128, cols), dtype=np.float16)
    j = np.arange(128)[:, None]
    for sp in scales:
        wl, wr, wi, kcut = sp["wl"], sp["wr"], sp["wi"], sp["kcut"]
        if sp["mode"] == "eo":
            u = np.arange(64)[None, :]
            k = wl - 64 * sp["G"] + j - u
            valid = (k >= 0) & (k < kcut)
            kc = np.clip(k, 0, wl - 1)
            blk = np.zeros((128, 128), dtype=np.float32)
            blk[:, 0::2] = np.where(valid, wr[kc], 0.0)
            blk[:, 1::2] = np.where(valid, wi[kc], 0.0)
            amat[:, sp["col"]:sp["col"] + 128] = blk.astype(np.float16)
            continue
        u = np.arange(128)[None, :]
        for i, t in enumerate(sp["ts"]):
            k = wl - sp["ph"] + j - u - 128 * t
            valid = (k >= 0) & (k < kcut)
            kc = np.clip(k, 0, wl - 1)
            blk = np.zeros((128, 256), dtype=np.float32)
            blk[:, 0::2] = np.where(valid, wr[kc], 0.0)
            blk[:, 1::2] = np.where(valid, wi[kc], 0.0)
            off = sp["col"] + i * 256
            amat[:, off:off + 256] = blk.astype(np.float16)
    return scales, amat


def _make_sig(sig_rows):
    """(ROWS, L) fp32 -> (ROWS, 2 phases, 128, PAD+NT) fp16 tiled/padded."""
    st = np.zeros((ROWS, 2, 128, PAD + NT), dtype=np.float16)
    s16 = sig_rows.astype(np.float16)
    for r in range(ROWS):
        st[r, 0, :, PAD:] = s16[r].reshape(NT, 128).T
        x64 = np.zeros(L, dtype=np.float16)
        x64[64:] = s16[r][:L - 64]
        st[r, 1, :, PAD:] = x64.reshape(NT, 128).T
    return st


def _build_nc(scales, acols):
    """Build + schedule + compile the per-core Bass program."""
    nc = bacc.Bacc("TRN2", target_bir_lowering=False, debug=False,
                   num_devices=NCORES)

    sig_d = nc.dram_tensor("sig", [ROWS, 2, 128, PAD + NT], F16,
                           kind="ExternalInput")
    amat_d = nc.dram_tensor("amat", [128, acols], F16, kind="ExternalInput")
    # out[row, half, c, s, 2u+comp] ; n = half*16384 + c*128 + u
    out_d = nc.dram_tensor("out", [ROWS, 2, 128, NSC, 256], F16,
                           kind="ExternalOutput")

    with tile.TileContext(nc) as tc:
        with tc.tile_pool(name="const", bufs=1) as const_pool, \
             tc.tile_pool(name="ob", bufs=8) as ob_pool, \
             tc.tile_pool(name="ps", bufs=1, space="PSUM") as ps_pool:

            # tiny ACT warm-up so the activation-table load (~2.7us)
            # happens under the input DMAs, not on the first real copy
            wz = const_pool.tile([128, 512], F16, tag="wz")
            wz2 = const_pool.tile([128, 8], F16, tag="wz2")
            nc.vector.memset(wz[:], 0)
            nc.scalar.copy(wz2[:], wz[:, 0:8])

            # amat chunked across BOTH HWDGE rings so the first matmuls only
            # wait for a small chunk; signal as one fused DMA on the sync ring
            amat_t = const_pool.tile([128, acols], F16, tag="amat")
            sig_all = const_pool.tile([128, 4 * (PAD + NT)], F16, tag="sig")

            def acol(s):
                return scales[s]["col"] if s < NSC else acols

            def amat_dma(eng, s0, s1):
                c0, c1 = acol(s0), acol(s1)
                eng.dma_start(out=amat_t[:, c0:c1],
                              in_=amat_d.ap()[:, c0:c1])

            amat_dma(nc.scalar, 0, 2)
            nc.scalar.dma_start(
                out=sig_all[:].rearrange("j (g m) -> j g m", g=4),
                in_=sig_d.ap().rearrange("r p j m -> j (r p) m"))
            amat_dma(nc.scalar, 2, 4)
            amat_dma(nc.scalar, 4, 8)
            amat_dma(nc.scalar, 8, 12)
            amat_dma(nc.scalar, 12, 16)

            def sig_slice(r, p, lo, hi):
                base = (2 * r + p) * (PAD + NT)
                return sig_all[:, base + lo:base + hi]

            # HAM warm-up: dummy matmuls keep the PE busy during the input
            # DMAs so real matmuls run at 2.4 GHz from the start
            dmy = ps_pool.tile([128, 2, 512], F32, tag="ps0")
            for _ in range(8):
                nc.tensor.matmul(dmy[:, 0, :], wz[:, 0:128], wz[:],
                                 start=True, stop=True)

            pg = 0
            for grp in range(2):
                for row in range(ROWS):
                    for half in range(2):
                        ob = ob_pool.tile([128, 8, 256], F16, tag="ob")
                        for pair in range(4):     # 2 scales per psum tile
                            pg += 1
                            ps = ps_pool.tile([128, 2, 512], F32,
                                              tag=f"ps{pg % 4}")
                            for kk in range(2):
                                s = grp * 8 + pair * 2 + kk
                                sp = scales[s]
                                if sp["mode"] == "eo":
                                    # even/odd half-tile: n = 128m + 64*eo + u
                                    for eo in range(2):
                                        Ge = sp["G"] - eo
                                        lo = PAD + 128 * half - (Ge - Ge % 2) // 2
                                        nc.tensor.matmul(
                                            ps[:, kk, eo * 128:eo * 128 + 128],
                                            sig_slice(row, Ge % 2,
                                                      lo, lo + 128),
                                            amat_t[:, sp["col"]:
                                                   sp["col"] + 128],
                                            start=True, stop=True,
                                        )
                                    continue
                                nts = len(sp["ts"])
                                for i, t in enumerate(sp["ts"]):
                                    lo = PAD + 128 * half - t
                                    nc.tensor.matmul(
                                        ps[:, kk, 0:256],
                                        sig_slice(row, sp["ph"] // 64,
                                                  lo, lo + 128),
                                        amat_t[:, sp["col"] + i * 256:
                                               sp["col"] + (i + 1) * 256],
                                        start=(i == 0),
                                        stop=(i == nts - 1),
                                    )
                            dst = ob[:, pair * 2:pair * 2 + 2, :]
                            src2 = ps[:, :, 0:256]
                            if pair % 2 == 0:
                                nc.vector.tensor_copy(dst, src2)
                            else:
                                nc.scalar.copy(dst, src2)
                        last = (grp == 1 and row == ROWS - 1 and half == 1)
                        if not last:
                            dma_eng = nc.sync if (row + half) % 2 == 0 \
                                else nc.scalar
                            dma_eng.dma_start(
                                out=out_d.ap()[row, half, :,
                                               grp * 8:(grp + 1) * 8, :]
                                    .rearrange("c s i -> c (s i)"),
                                in_=ob[:].rearrange("c s i -> c (s i)"),
                            )
                        else:
                            # final store split across both rings so the
                            # kernel tail is half the size
                            for q, eng in ((0, nc.sync), (1, nc.scalar)):
                                s0 = grp * 8 + q * 4
                                eng.dma_start(
                                    out=out_d.ap()[row, half, :,
                                                   s0:s0 + 4, :]
                                        .rearrange("c s i -> c (s i)"),
                                    in_=ob[:, q * 4:(q + 1) * 4, :]
                                        .rearrange("c s i -> c (s i)"),
                                )
    nc.compile()
    return nc


_CACHE = {}


def _get_nc(key, scales, acols):
    if key not in _CACHE:
        _CACHE[key] = _build_nc(scales, acols)
    return _CACHE[key]


def kernel(signal, scales_log, center_freq_log, bandwidth_log):
    signal = np.asarray(signal, dtype=np.float32)
    cf = np.float32(np.exp(np.float32(np.asarray(center_freq_log))))
    bw = np.float32(np.exp(np.float32(np.asarray(bandwidth_log))))

    scales, amat = _plan(cf, bw)
    key = tuple((sp["mode"], sp["col"], sp.get("G", -1),
                 tuple(sp.get("ts", ())), sp.get("ph", -1))
                for sp in scales)
    nc = _get_nc(key, scales, amat.shape[1])

    # host: per-core transposed/padded fp16 signal
    in_maps = []
    for core in range(NCORES):
        st = _make_sig(signal[core * ROWS:(core + 1) * ROWS])
        in_maps.append({"sig": st, "amat": amat})

    res = run_bass_kernel_spmd(nc, in_maps, core_ids=list(range(NCORES)))

    out = np.empty((B, NSC, L), dtype=np.complex64)
    for core in range(NCORES):
        o = np.asarray(res.results[core]["out"], dtype=np.float32)
        # [row, half, c, s, 2u+comp] -> [row, s, half, c, u, comp]
        o = o.transpose(0, 3, 1, 2, 4).reshape(ROWS, NSC, L, 2)
        out[core * ROWS:(core + 1) * ROWS] = o[..., 0] + 1j * o[..., 1]
    return out


# revision 22
# speedup vs baseline: 1.1565x; 1.0784x over previous
"""Trainium2 Bass kernel for nn_AdaptiveWaveletBank.

out[b, s, n] = sum_k w_s[k] * signal[b, n - wl_s + k]   (complex w, zero-pad)

Strategy:
  - Data-parallel over batch: 16 rows -> 8 cores x 2 rows.
  - The Morlet-like wavelet w_s decays as exp(-0.5 (k/scale)^2): only the
    first ~6.1*scale taps matter (<1e-8 of peak).  Host truncates.
  - Per core the conv is expressed as banded matmuls on the TensorEngine:
    signal laid out as tiles S[j, m] = sig[m*128 + j]; for each output
    half (128 tiles x 128 samples) and scale, accumulate over tile-shifts t:
        out[c, 2u+comp] += sum_j S[j, c + c0 - t] * A_{s,t}[j, 2u+comp]
    with A_{s,t}[j, 2u+comp] = w_comp[wl_s + j - u - 128 t] (banded Toeplitz,
    built on host, fp16).  PSUM accumulates in fp32; DVE copies/casts to an
    fp16 staging buffer laid out so the output DMA is fully contiguous.
  - Host pre-transposes/pads the signal and assembles complex64 output.
"""

import numpy as np

import concourse.bacc as bacc
import concourse.bass as bass
import concourse.mybir as mybir
import concourse.tile as tile
from concourse.bass_utils import run_bass_kernel_spmd

B, L, NSC = 16, 32768, 16
NCORES = 8
ROWS = B // NCORES          # rows of the batch per core
NT = L // 128               # 256 signal tiles of 128 samples
PAD = 16                    # leading zero tiles (max tile shift t = 16)
NUM_OSC = 6.0
ENV_CUT = 1e-8              # truncate wavelet where envelope < this

F16 = mybir.dt.float16
F32 = mybir.dt.float32


def _scales_and_lengths():
    s = np.exp(np.linspace(np.log(1.0), np.log(32.0), NSC))
    lengths = []
    for sc in s:
        wl = min(int(L * 0.5), int(64 * sc))
        wl = max(wl, 8)
        wl = wl if wl % 2 == 0 else wl + 1
        lengths.append(wl)
    return s, lengths


def _wavelets(sc, wl, cf, bw):
    # float32 arithmetic to mirror the jnp reference
    t = np.arange(wl, dtype=np.float32) / (bw * np.float32(max(float(sc), 0.1)))
    env = np.exp(-0.5 * t * t).astype(np.float32)
    ph = (np.float32(2.0 * np.pi / NUM_OSC) * cf * t).astype(np.float32)
    wr = env * np.cos(ph)
    wi = env * np.sin(ph)
    norm = np.max(np.sqrt(wr * wr + wi * wi)) + np.float32(1e-8)
    return (wr / norm).astype(np.float32), (wi / norm).astype(np.float32), env


def _plan(cf, bw):
    """Per-scale wavelets, truncation and tile-shift ranges + packed A matrix.

    Each scale picks a signal-tiling phase (0 or 64 samples) minimizing the
    number of tile-shift matmuls nT."""
    s_vals, wlens = _scales_and_lengths()
    scales = []
    cols = 0
    for sc, wl in zip(s_vals, wlens):
        wr, wi, env = _wavelets(sc, wl, cf, bw)
        kcut = int(np.sum(env > ENV_CUT))
        kcut = max(1, min(kcut, wl))
        pad64 = (64 - wl % 64) % 64
        if pad64 + kcut <= 64 and wl >= 64:
            # even/odd half-tile mode: two 128-col single matmuls sharing
            # one A block; G = ceil(wl/64) aligns the 128-sample window
            scales.append(dict(wl=wl, wr=wr, wi=wi, kcut=kcut, mode="eo",
                               G=-(-wl // 64), col=cols))
            cols += 128
            continue
        best = None
        for ph in (0, 64):
            t_hi = (wl - ph + 127) // 128
            t_lo = -(-(wl - ph - kcut - 126) // 128)
            if t_lo < 0 and ph > 0:
                continue          # would need a forward tile shift
            t_lo = max(0, t_lo)
            if best is None or t_hi - t_lo < best[1] - best[0]:
                best = (t_lo, t_hi, ph)
        t_lo, t_hi, ph = best
        ts = list(range(t_lo, t_hi + 1))
        scales.append(dict(wl=wl, wr=wr, wi=wi, kcut=kcut, mode="chain",
                           ts=ts, col=cols, ph=ph))
        cols += len(ts) * 256

    amat = np.zeros((128, cols), dtype=np.float16)
    j = np.arange(128)[:, None]
    for sp in scales:
        wl, wr, wi, kcut = sp["wl"], sp["wr"], sp["wi"], sp["kcut"]
        if sp["mode"] == "eo":
            u = np.arange(64)[None, :]
            k = wl - 64 * sp["G"] + j - u
            valid = (k >= 0) & (k < kcut)
            kc = np.clip(k, 0, wl - 1)
            blk = np.zeros((128, 128), dtype=np.float32)
            blk[:, 0::2] = np.where(valid, wr[kc], 0.0)
            blk[:, 1::2] = np.where(valid, wi[kc], 0.0)
            amat[:, sp["col"]:sp["col"] + 128] = blk.astype(np.float16)
            continue
        u = np.arange(128)[None, :]
        for i, t in enumerate(sp["ts"]):
            k = wl - sp["ph"] + j - u - 128 * t
            valid = (k >= 0) & (k < kcut)
            kc = np.clip(k, 0, wl - 1)
            blk = np.zeros((128, 256), dtype=np.float32)
            blk[:, 0::2] = np.where(valid, wr[kc], 0.0)
            blk[:, 1::2] = np.where(valid, wi[kc], 0.0)
            off = sp["col"] + i * 256
            amat[:, off:off + 256] = blk.astype(np.float16)
    return scales, amat


def _make_sig(sig_rows):
    """(ROWS, L) fp32 -> (ROWS, 2 phases, 128, PAD+NT) fp16 tiled/padded."""
    st = np.zeros((ROWS, 2, 128, PAD + NT), dtype=np.float16)
    s16 = sig_rows.astype(np.float16)
    for r in range(ROWS):
        st[r, 0, :, PAD:] = s16[r].reshape(NT, 128).T
        x64 = np.zeros(L, dtype=np.float16)
        x64[64:] = s16[r][:L - 64]
        st[r, 1, :, PAD:] = x64.reshape(NT, 128).T
    return st


def _build_nc(scales, acols):
    """Build + schedule + compile the per-core Bass program."""
    nc = bacc.Bacc("TRN2", target_bir_lowering=False, debug=False,
                   num_devices=NCORES)

    sig_d = nc.dram_tensor("sig", [ROWS, 2, 128, PAD + NT], F16,
                           kind="ExternalInput")
    amat_d = nc.dram_tensor("amat", [128, acols], F16, kind="ExternalInput")
    # out[row, half, c, s, 2u+comp] ; n = half*16384 + c*128 + u
    out_d = nc.dram_tensor("out", [ROWS, 2, 128, NSC, 256], F16,
                           kind="ExternalOutput")

    with tile.TileContext(nc) as tc:
        with tc.tile_pool(name="const", bufs=1) as const_pool, \
             tc.tile_pool(name="ob", bufs=16) as ob_pool, \
             tc.tile_pool(name="ps", bufs=1, space="PSUM") as ps_pool:

            # tiny ACT warm-up so the activation-table load (~2.7us)
            # happens under the input DMAs, not on the first real copy
            wz = const_pool.tile([128, 512], F16, tag="wz")
            wz2 = const_pool.tile([128, 8], F16, tag="wz2")
            nc.gpsimd.memset(wz[:], 0)
            nc.scalar.copy(wz2[:], wz[:, 0:8])

            # amat chunked across BOTH HWDGE rings so the first matmuls only
            # wait for a small chunk; signal as one fused DMA on the sync ring
            amat_t = const_pool.tile([128, acols], F16, tag="amat")
            sig_all = const_pool.tile([128, 4 * (PAD + NT)], F16, tag="sig")

            def acol(s):
                return scales[s]["col"] if s < NSC else acols

            def amat_dma(eng, s0, s1):
                c0, c1 = acol(s0), acol(s1)
                eng.dma_start(out=amat_t[:, c0:c1],
                              in_=amat_d.ap()[:, c0:c1])

            amat_dma(nc.scalar, 0, 2)
            nc.scalar.dma_start(
                out=sig_all[:].rearrange("j (g m) -> j g m", g=4),
                in_=sig_d.ap().rearrange("r p j m -> j (r p) m"))
            amat_dma(nc.scalar, 2, 4)
            amat_dma(nc.scalar, 4, 8)
            amat_dma(nc.scalar, 8, 12)
            amat_dma(nc.scalar, 12, 16)

            def sig_slice(r, p, lo, hi):
                base = (2 * r + p) * (PAD + NT)
                return sig_all[:, base + lo:base + hi]

            # HAM warm-up: dummy matmuls keep the PE busy during the input
            # DMAs so real matmuls run at 2.4 GHz from the start
            dmy = ps_pool.tile([128, 2, 512], F32, tag="ps0")
            for _ in range(8):
                nc.tensor.matmul(dmy[:, 0, :], wz[:, 0:128], wz[:],
                                 start=True, stop=True)

            pg = 0
            for grp in range(2):
                for row in range(ROWS):
                    for half in range(2):
                        ob = ob_pool.tile([128, 8, 256], F16, tag="ob")
                        for pair in range(4):     # 2 scales per psum tile
                            pg += 1
                            ps = ps_pool.tile([128, 2, 512], F32,
                                              tag=f"ps{pg % 4}")
                            for kk in range(2):
                                s = grp * 8 + pair * 2 + kk
                                sp = scales[s]
                                if sp["mode"] == "eo":
                                    # even/odd half-tile: n = 128m + 64*eo + u
                                    for eo in range(2):
                                        Ge = sp["G"] - eo
                                        lo = PAD + 128 * half - (Ge - Ge % 2) // 2
                                        nc.tensor.matmul(
                                            ps[:, kk, eo * 128:eo * 128 + 128],
                                            sig_slice(row, Ge % 2,
                                                      lo, lo + 128),
                                            amat_t[:, sp["col"]:
                                                   sp["col"] + 128],
                                            start=True, stop=True,
                                        )
                                    continue
                                nts = len(sp["ts"])
                                for i, t in enumerate(sp["ts"]):
                                    lo = PAD + 128 * half - t
                                    nc.tensor.matmul(
                                        ps[:, kk, 0:256],
                                        sig_slice(row, sp["ph"] // 64,
                                                  lo, lo + 128),
                                        amat_t[:, sp["col"] + i * 256:
                                               sp["col"] + (i + 1) * 256],
                                        start=(i == 0),
                                        stop=(i == nts - 1),
                                    )
                            dst = ob[:, pair * 2:pair * 2 + 2, :]
                            src2 = ps[:, :, 0:256]
                            if pair < 2:
                                nc.scalar.copy(dst, src2)
                            else:
                                nc.vector.tensor_copy(dst, src2)
                        last = (grp == 1 and row == ROWS - 1 and half == 1)
                        if not last:
                            dma_eng = nc.sync if (row + half) % 2 == 0 \
                                else nc.scalar
                            dma_eng.dma_start(
                                out=out_d.ap()[row, half, :,
                                               grp * 8:(grp + 1) * 8, :]
                                    .rearrange("c s i -> c (s i)"),
                                in_=ob[:].rearrange("c s i -> c (s i)"),
                            )
                        else:
                            # final store split across both rings so the
                            # kernel tail is half the size
                            for q, eng in ((0, nc.sync), (1, nc.scalar)):
                                s0 = grp * 8 + q * 4
                                eng.dma_start(
                                    out=out_d.ap()[row, half, :,
                                                   s0:s0 + 4, :]
                                        .rearrange("c s i -> c (s i)"),
                                    in_=ob[:, q * 4:(q + 1) * 4, :]
                                        .rearrange("c s i -> c (s i)"),
                                )
    nc.compile()
    return nc


_CACHE = {}


def _get_nc(key, scales, acols):
    if key not in _CACHE:
        _CACHE[key] = _build_nc(scales, acols)
    return _CACHE[key]


def kernel(signal, scales_log, center_freq_log, bandwidth_log):
    signal = np.asarray(signal, dtype=np.float32)
    cf = np.float32(np.exp(np.float32(np.asarray(center_freq_log))))
    bw = np.float32(np.exp(np.float32(np.asarray(bandwidth_log))))

    scales, amat = _plan(cf, bw)
    key = tuple((sp["mode"], sp["col"], sp.get("G", -1),
                 tuple(sp.get("ts", ())), sp.get("ph", -1))
                for sp in scales)
    nc = _get_nc(key, scales, amat.shape[1])

    # host: per-core transposed/padded fp16 signal
    in_maps = []
    for core in range(NCORES):
        st = _make_sig(signal[core * ROWS:(core + 1) * ROWS])
        in_maps.append({"sig": st, "amat": amat})

    res = run_bass_kernel_spmd(nc, in_maps, core_ids=list(range(NCORES)))

    out = np.empty((B, NSC, L), dtype=np.complex64)
    for core in range(NCORES):
        o = np.asarray(res.results[core]["out"], dtype=np.float32)
        # [row, half, c, s, 2u+comp] -> [row, s, half, c, u, comp]
        o = o.transpose(0, 3, 1, 2, 4).reshape(ROWS, NSC, L, 2)
        out[core * ROWS:(core + 1) * ROWS] = o[..., 0] + 1j * o[..., 1]
    return out
